# revision 7
# baseline (speedup 1.0000x reference)
"""Trainium2 Bass kernel for ViT-style multi-head attention with relative
position bias.

Problem (per full input):
  x        [8, 1024, 768] f32
  W_qkv    [768, 2304]    f32
  W_proj   [768, 768]     f32
  b_proj   [768]          f32
  bias_table [2047, 12]   f32
  rel_index  [1024, 1024] int32

Sharding: pure data parallel — one batch element per NeuronCore (B=8 over 8
cores), weights replicated. No collectives.

Per-core kernel layout strategy (all matmuls bf16, accum f32 in PSUM):
  - host pre-transposes x -> xT [C, N] so both qkv orientations work:
      qT,kT [d, n] = W_qk^T-style matmul (lhsT = W cols, rhs = xT)
      v     [n, d] = (lhsT = xT cols,  rhs = W_v)
  - scores computed TRANSPOSED  sT[j, i] = kT^T-chunk @ qT  so that the
    softmax'd matrix pT is already in the [j(part), i(free)] layout the PV
    matmul needs as its moving operand.
  - relative-position bias folded in multiplicatively: host precomputes
    E = exp(biasT) and the kernel computes pT = exp(sT) * E  (exp on ScalarE,
    multiply on VectorE in bf16).
  - PV uses v augmented with a ones column: out'T[65, i] = [v|1]^T @ pT —
    row 64 is the softmax denominator for free.
  - normalization: reciprocal of the denominator row, broadcast across 64
    partitions, multiply during PSUM->SBUF eviction straight into outT layout.
  - proj: lhsT = outT chunks (exactly the layout produced), rhs = W_proj,
    + broadcast b_proj, DMA out.

K=64 score matmuls for the two heads of a pair are issued back-to-back at PE
row groups 0 and 64 (auto tile_position) so they run concurrently in the
128x128 array.
"""

import numpy as np
import ml_dtypes

# ---- problem constants (hardcoded per task rules) ----
B = 8
N = 1024
C = 768
H = 12
DH = 64
P = 128
KC = C // P          # 6 contraction chunks of 128 over C
NJ = N // P          # 8 chunks of 128 over the j (key) axis
NT = N // 512        # 2 tiles of 512 over the i (query) axis
HP = H // 2          # 6 head pairs
T5 = 512

# broadcast strategy for the softmax denominator row:
#   "dma"    - SBUF->SBUF DMA with partition-step-0 source AP
#   "matmul" - K=1 ones outer-product matmul + DMA copy of the PSUM result
BCAST = "dma"

_BUILt = {}


def _build_nc():
    from contextlib import ExitStack
    import concourse.bass as bass
    import concourse.mybir as mybir
    import concourse.tile as tile
    from concourse import bacc

    bf16 = mybir.dt.bfloat16
    f32 = mybir.dt.float32
    Exp = mybir.ActivationFunctionType.Exp

    nc = bacc.Bacc("TRN2", target_bir_lowering=False, debug=False)

    xT_d = nc.dram_tensor("xT", [C, N], bf16, kind="ExternalInput")
    w_d = nc.dram_tensor("wqkv", [C, 3 * C], bf16, kind="ExternalInput")
    wp_d = nc.dram_tensor("wproj", [C, C], bf16, kind="ExternalInput")
    bp_d = nc.dram_tensor("bproj", [C], f32, kind="ExternalInput")
    # E = exp(rel bias), transposed + pair-tiled: [hp, jc, p(j), t(head in pair), i]
    be_d = nc.dram_tensor("bexp", [HP, NJ, P, 2, N], bf16, kind="ExternalInput")
    out_d = nc.dram_tensor("out", [N, C], f32, kind="ExternalOutput")

    with ExitStack() as ctx:
        tc = ctx.enter_context(tile.TileContext(nc))

        singles = ctx.enter_context(tc.tile_pool(name="singles", bufs=1))
        pt_pool = ctx.enter_context(tc.tile_pool(name="pt_pool", bufs=2))
        e_pool = ctx.enter_context(tc.tile_pool(name="e_pool", bufs=3))
        rec_pool = ctx.enter_context(tc.tile_pool(name="rec_pool", bufs=2))
        bc_pool = ctx.enter_context(tc.tile_pool(name="bc_pool", bufs=2))
        stg_pool = ctx.enter_context(tc.tile_pool(name="stg_pool", bufs=2))
        ost_pool = ctx.enter_context(tc.tile_pool(name="ost_pool", bufs=2))
        o_pool = ctx.enter_context(tc.tile_pool(name="o_pool", bufs=3))
        dram_pool = ctx.enter_context(tc.tile_pool(name="dram_pool", bufs=2, space="DRAM"))
        mm_ps = ctx.enter_context(tc.tile_pool(name="mm_ps", bufs=2, space="PSUM"))
        sc_ps = ctx.enter_context(tc.tile_pool(name="sc_ps", bufs=1, space="PSUM"))
        pv_ps = ctx.enter_context(tc.tile_pool(name="pv_ps", bufs=2, space="PSUM"))

        # ---- resident SBUF tensors ----
        xT_sb = singles.tile([P, KC, N], bf16)
        nc.sync.dma_start(out=xT_sb, in_=xT_d.ap().rearrange("(kc p) n -> p kc n", p=P))
        w_sb = singles.tile([P, KC, 3 * C], bf16)
        nc.sync.dma_start(out=w_sb, in_=w_d.ap().rearrange("(kc p) d -> p kc d", p=P))
        wp_sb = singles.tile([P, KC, C], bf16)
        nc.sync.dma_start(out=wp_sb, in_=wp_d.ap().rearrange("(kc p) d -> p kc d", p=P))
        bp_sb = singles.tile([P, C], f32)
        bp_ap = bp_d.ap()
        bp_bcast = bass.AP(tensor=bp_ap.tensor, offset=bp_ap.offset,
                           ap=[[0, P], *bp_ap.ap])
        nc.gpsimd.dma_start(out=bp_sb, in_=bp_bcast)

        qT_sb = singles.tile([P, KC, N], bf16)   # chunk hp = heads (2hp, 2hp+1)
        kT_sb = singles.tile([P, KC, N], bf16)
        v_sb = singles.tile([P, NJ, H, DH + 1], bf16)  # col DH = ones
        nc.vector.memset(v_sb[:, :, :, DH:DH + 1], 1.0)
        outT_sb = singles.tile([P, KC, N], bf16)

        # ---- v = x @ W_v  ->  v_sb [n(part), d] per head, + ones col ----
        with nc.named_scope("v_mm"):
            for nj in range(NJ):
                for e0 in (0, 384):
                    ps = mm_ps.tile([P, T5], mybir.dt.float32, tag="mm")
                    for kc in range(KC):
                        nc.tensor.matmul(
                            ps[:, :384],
                            xT_sb[:, kc, nj * P:(nj + 1) * P],
                            w_sb[:, kc, 2 * C + e0:2 * C + e0 + 384],
                            start=(kc == 0), stop=(kc == KC - 1),
                        )
                    for h in range(e0 // DH, (e0 + 384) // DH):
                        nc.vector.tensor_copy(
                            out=v_sb[:, nj, h, 0:DH],
                            in_=ps[:, h * DH - e0:h * DH - e0 + DH],
                        )

        for hp in range(HP):
            # ---- qT / kT chunks for this head pair ----
            with nc.named_scope("qk_mm"):
                for dst, col0 in ((qT_sb, hp * P), (kT_sb, C + hp * P)):
                    for it in range(NT):
                        ps = mm_ps.tile([P, T5], mybir.dt.float32, tag="mm")
                        for kc in range(KC):
                            nc.tensor.matmul(
                                ps,
                                w_sb[:, kc, col0:col0 + P],
                                xT_sb[:, kc, it * T5:(it + 1) * T5],
                                start=(kc == 0), stop=(kc == KC - 1),
                            )
                        nc.vector.tensor_copy(
                            out=dst[:, hp, it * T5:(it + 1) * T5], in_=ps)

            # ---- scores (transposed) + exp + bias-multiply -> pT ----
            pt = pt_pool.tile([P, NJ, 2, N], bf16, tag="pt")
            with nc.named_scope("scores"):
                for jc in range(NJ):
                    e_t = e_pool.tile([P, 2, N], bf16, tag="e")
                    nc.sync.dma_start(out=e_t, in_=be_d.ap()[hp, jc])
                    sc = sc_ps.tile([P, 2 * N], mybir.dt.float32, tag="sc")
                    for it in range(NT):
                        # two heads of the pair at PE row groups 0 / 64 (concurrent)
                        nc.tensor.matmul(
                            sc[:, 0 * N + it * T5:0 * N + (it + 1) * T5],
                            kT_sb[0:64, hp, jc * P:(jc + 1) * P],
                            qT_sb[0:64, hp, it * T5:(it + 1) * T5],
                            start=True, stop=True,
                        )
                        nc.tensor.matmul(
                            sc[:, 1 * N + it * T5:1 * N + (it + 1) * T5],
                            kT_sb[64:128, hp, jc * P:(jc + 1) * P],
                            qT_sb[64:128, hp, it * T5:(it + 1) * T5],
                            start=True, stop=True,
                        )
                    nc.scalar.activation(
                        out=pt[:, jc],
                        in_=sc[:].rearrange("p (t n) -> p t n", t=2),
                        func=Exp,
                    )
                    nc.vector.tensor_mul(out=pt[:, jc], in0=pt[:, jc], in1=e_t)

            # ---- PV + softmax normalize -> outT_sb ----
            with nc.named_scope("pv"):
                for t in range(2):
                    h = 2 * hp + t
                    for it in range(NT):
                        pv = pv_ps.tile([DH + 1, T5], mybir.dt.float32, tag="pv")
                        for jc in range(NJ):
                            nc.tensor.matmul(
                                pv,
                                v_sb[:, jc, h, :],
                                pt[:, jc, t, it * T5:(it + 1) * T5],
                                start=(jc == 0), stop=(jc == NJ - 1),
                            )
                        rec = rec_pool.tile([DH + 1, T5], bf16, tag="rec")
                        with nc.allow_low_precision(reason="bf16 softmax denom"):
                            nc.vector.reciprocal(
                                out=rec[DH:DH + 1, :], in_=pv[DH:DH + 1, :])
                        # evict PV result early so the PSUM slot frees fast
                        o_sb = o_pool.tile([DH, T5], bf16, tag="o")
                        nc.vector.tensor_copy(out=o_sb, in_=pv[0:DH, :])
                        # broadcast the reciprocal row across 64 partitions via
                        # a DRAM bounce (step-0 partition APs only work on DRAM)
                        dr = dram_pool.tile([1, T5], bf16, tag="dr")
                        nc.sync.dma_start(out=dr, in_=rec[DH:DH + 1, :])
                        bc = bc_pool.tile([DH, T5], bf16, tag="bc")
                        bsrc = bass.AP(tensor=dr.tensor, offset=dr.offset,
                                       ap=[[0, DH], [1, T5]])
                        nc.sync.dma_start(out=bc, in_=bsrc)
                        if t == 0:
                            nc.vector.tensor_mul(
                                out=outT_sb[0:DH, hp, it * T5:(it + 1) * T5],
                                in0=o_sb, in1=bc)
                        else:
                            st = stg_pool.tile([DH, T5], bf16, tag="st")
                            nc.vector.tensor_mul(out=st, in0=o_sb, in1=bc)
                            nc.sync.dma_start(
                                out=outT_sb[DH:P, hp, it * T5:(it + 1) * T5],
                                in_=st)

        # ---- proj + bias + DMA out ----
        with nc.named_scope("proj"):
            for nj in range(NJ):
                osb = ost_pool.tile([P, C], mybir.dt.float32, tag="osb")
                for et in range(2):
                    pp = mm_ps.tile([P, T5], mybir.dt.float32, tag="mm")
                    for kc in range(KC):
                        nc.tensor.matmul(
                            pp[:, :384],
                            outT_sb[:, kc, nj * P:(nj + 1) * P],
                            wp_sb[:, kc, et * 384:(et + 1) * 384],
                            start=(kc == 0), stop=(kc == KC - 1),
                        )
                    nc.vector.tensor_add(
                        out=osb[:, et * 384:(et + 1) * 384],
                        in0=pp[:, :384],
                        in1=bp_sb[:, et * 384:(et + 1) * 384],
                    )
                nc.sync.dma_start(out=out_d.ap()[nj * P:(nj + 1) * P, :], in_=osb)

    nc.finalize()
    return nc


def _get_nc():
    if "nc" not in _BUILt:
        _BUILt["nc"] = _build_nc()
    return _BUILt["nc"]


def _prep_inputs(x, W_qkv, W_proj, b_proj, bias_table, rel_index):
    bf = ml_dtypes.bfloat16
    x = np.asarray(x, dtype=np.float32)
    W_qkv = np.asarray(W_qkv, dtype=np.float32)
    W_proj = np.asarray(W_proj, dtype=np.float32)
    b_proj = np.asarray(b_proj, dtype=np.float32)
    bias_table = np.asarray(bias_table, dtype=np.float32)
    rel_index = np.asarray(rel_index)

    xT = np.ascontiguousarray(x.transpose(0, 2, 1)).astype(bf)       # [B, C, N]
    wq = W_qkv.copy()
    wq[:, :C] *= DH ** -0.5          # fold the attention scale into W_q
    wq = wq.astype(bf)
    wp = W_proj.astype(bf)

    # E[h, j, i] = exp(bias_table[rel_index[i, j], h]); pair-tiled layout
    g = bias_table[rel_index]                      # [i, j, H]
    E = np.exp(g).transpose(2, 1, 0)               # [H, j, i]
    Ep = E.reshape(HP, 2, NJ, P, N).transpose(0, 2, 3, 1, 4)  # [hp, jc, p, t, i]
    Ep = np.ascontiguousarray(Ep).astype(bf)

    shared = {"wqkv": wq, "wproj": wp, "bproj": b_proj, "bexp": Ep}
    in_maps = []
    for b in range(B):
        m = dict(shared)
        m["xT"] = np.ascontiguousarray(xT[b])
        in_maps.append(m)
    return in_maps


def run(x, W_qkv, W_proj, b_proj, bias_table, rel_index, trace=False):
    """Returns (output [B, N, C] f32, exec_time_ns or None)."""
    from concourse.bass_utils import run_bass_kernel_spmd

    nc = _get_nc()
    in_maps = _prep_inputs(x, W_qkv, W_proj, b_proj, bias_table, rel_index)
    res = run_bass_kernel_spmd(nc, in_maps, core_ids=list(range(B)), trace=trace)
    out = np.stack([r["out"] for r in res.results]).astype(np.float32)
    return out, res.exec_time_ns


def kernel(x, W_qkv, W_proj, b_proj, bias_table, rel_index):
    out, _ = run(x, W_qkv, W_proj, b_proj, bias_table, rel_index, trace=False)
    return out


# revision 14
# speedup vs baseline: 1.0795x; 1.0795x over previous
"""Trainium2 Bass kernel for ViT-style multi-head attention with relative
position bias.

Problem (per full input):
  x        [8, 1024, 768] f32
  W_qkv    [768, 2304]    f32
  W_proj   [768, 768]     f32
  b_proj   [768]          f32
  bias_table [2047, 12]   f32
  rel_index  [1024, 1024] int32

Sharding: pure data parallel — one batch element per NeuronCore (B=8 over 8
cores), weights replicated. No collectives.

Per-core kernel (all matmuls bf16, accum f32 in PSUM):
  - host pre-transposes x -> xT [C, N]; qT,kT computed in [d, n] layout,
    v in [n, d] layout — both directly from xT, no on-device transposes.
  - scores computed TRANSPOSED sT[j, i] = kT_chunk^T @ qT so the softmax'd
    matrix pT is already the PV matmul's moving operand. The two heads of a
    pair run concurrently at PE row groups 0/64 (K=64 row tiling).
  - rel-pos bias folded multiplicatively: host precomputes E = exp(biasT);
    kernel does pT = exp(sT) * E (exp on ScalarE over [128, 2048] pair
    tiles, multiply on VectorE).
  - PV: out'T[65, i] = [v|1]^T @ pT — row 64 is the softmax denominator.
  - normalize: reciprocal_approx_fast of the denom row, broadcast across 64
    partitions via a DRAM bounce (step-0 partition APs are DRAM-only),
    multiply during PSUM->SBUF eviction into the outT layout proj needs.
  - proj: lhsT = outT chunks, + broadcast b_proj, DMA out.

Program order interleaves qkv / v / PV matmul groups into the score slots so
the PE never idles while ScalarE runs exp — keeps the HAM clock gate at
2.4 GHz (idle windows drop the PE to 1.2 GHz).
"""

import numpy as np
import ml_dtypes

B = 8
N = 1024
C = 768
H = 12
DH = 64
P = 128
KC = C // P          # 6 contraction chunks of 128 over C
NJ = N // P          # 8 chunks of 128 over the j (key) axis
NT = N // 512        # 2 tiles of 512 over the i (query) axis
HP = H // 2          # 6 head pairs
T5 = 512

_BUILT = {}


def _build_nc():
    from contextlib import ExitStack
    import concourse.bass as bass
    import concourse.mybir as mybir
    import concourse.tile as tile
    from concourse import bacc

    bf16 = mybir.dt.bfloat16
    f32 = mybir.dt.float32
    Exp = mybir.ActivationFunctionType.Exp

    nc = bacc.Bacc("TRN2", target_bir_lowering=False, debug=False)

    xT_d = nc.dram_tensor("xT", [C, N], bf16, kind="ExternalInput")
    w_d = nc.dram_tensor("wqkv", [C, 3 * C], bf16, kind="ExternalInput")
    wp_d = nc.dram_tensor("wproj", [C, C], bf16, kind="ExternalInput")
    bp_d = nc.dram_tensor("bproj", [C], f32, kind="ExternalInput")
    be_d = nc.dram_tensor("bexp", [HP, NJ, P, 2, N], bf16, kind="ExternalInput")
    out_d = nc.dram_tensor("out", [N, C], f32, kind="ExternalOutput")

    with ExitStack() as ctx:
        tc = ctx.enter_context(tile.TileContext(nc))

        singles = ctx.enter_context(tc.tile_pool(name="singles", bufs=1))
        pt_pool = ctx.enter_context(tc.tile_pool(name="pt_pool", bufs=2))
        e_pool = ctx.enter_context(tc.tile_pool(name="e_pool", bufs=3))
        rec_pool = ctx.enter_context(tc.tile_pool(name="rec_pool", bufs=2))
        bc_pool = ctx.enter_context(tc.tile_pool(name="bc_pool", bufs=2))
        stg_pool = ctx.enter_context(tc.tile_pool(name="stg_pool", bufs=2))
        ost_pool = ctx.enter_context(tc.tile_pool(name="ost_pool", bufs=2))
        dram_pool = ctx.enter_context(tc.tile_pool(name="dram_pool", bufs=2, space="DRAM"))
        mm_ps = ctx.enter_context(tc.tile_pool(name="mm_ps", bufs=2, space="PSUM"))
        sc_ps = ctx.enter_context(tc.tile_pool(name="sc_ps", bufs=1, space="PSUM"))
        pv_ps = ctx.enter_context(tc.tile_pool(name="pv_ps", bufs=2, space="PSUM"))

        # ---- resident SBUF tensors (loads chunked so the PE starts early) --
        xT_sb = singles.tile([P, KC, N], bf16)
        xT_r = xT_d.ap().rearrange("(kc p) n -> p kc n", p=P)
        w_sb = singles.tile([P, KC, 3 * C], bf16)
        w_r = w_d.ap().rearrange("(kc p) d -> p kc d", p=P)
        for kc in range(KC):
            nc.sync.dma_start(out=xT_sb[:, kc], in_=xT_r[:, kc])
            nc.sync.dma_start(out=w_sb[:, kc], in_=w_r[:, kc])
        wp_sb = singles.tile([P, KC, C], bf16)
        nc.sync.dma_start(out=wp_sb, in_=wp_d.ap().rearrange("(kc p) d -> p kc d", p=P))
        bp_sb = singles.tile([P, C], f32)
        bp_ap = bp_d.ap()
        bp_bcast = bass.AP(tensor=bp_ap.tensor, offset=bp_ap.offset,
                           ap=[[0, P], *bp_ap.ap])
        nc.gpsimd.dma_start(out=bp_sb, in_=bp_bcast)

        qT_sb = singles.tile([P, KC, N], bf16)   # chunk hp = heads (2hp, 2hp+1)
        kT_sb = singles.tile([P, KC, N], bf16)
        v_sb = singles.tile([P, NJ, H, DH + 1], bf16)  # col DH = ones
        nc.vector.memset(v_sb[:, :, :, DH:DH + 1], 1.0)
        outT_sb = singles.tile([P, KC, N], bf16)

        # ---- matmul group emitters (closures; emitted in interleaved order) --

        def v_group(nj, et):
            e0 = et * 384
            def emit():
                with nc.named_scope("v_mm"):
                    ps = mm_ps.tile([P, T5], f32, tag="mm", name=f"ps_v_{nj}_{et}")
                    for kc in range(KC):
                        nc.tensor.matmul(
                            ps[:, :384],
                            xT_sb[:, kc, nj * P:(nj + 1) * P],
                            w_sb[:, kc, 2 * C + e0:2 * C + e0 + 384],
                            start=(kc == 0), stop=(kc == KC - 1),
                        )
                    h0 = e0 // DH
                    nc.vector.tensor_copy(
                        out=v_sb[:, nj, h0:h0 + 6, 0:DH],
                        in_=ps[:, :384].rearrange("p (h d) -> p h d", h=6),
                    )
            return emit

        def qk_group(hp, which, it):
            col0 = hp * P if which == 0 else C + hp * P
            dst = None
            def emit():
                with nc.named_scope("qk_mm"):
                    d = qT_sb if which == 0 else kT_sb
                    ps = mm_ps.tile([P, T5], f32, tag="mm",
                                    name=f"ps_qk_{hp}_{which}_{it}")
                    for kc in range(KC):
                        nc.tensor.matmul(
                            ps,
                            w_sb[:, kc, col0:col0 + P],
                            xT_sb[:, kc, it * T5:(it + 1) * T5],
                            start=(kc == 0), stop=(kc == KC - 1),
                        )
                    nc.vector.tensor_copy(
                        out=d[:, hp, it * T5:(it + 1) * T5], in_=ps)
            return emit

        def pv_group(hp, pt, t, it):
            def emit():
                with nc.named_scope("pv"):
                    h = 2 * hp + t
                    pv = pv_ps.tile([DH + 1, T5], f32, tag="pv",
                                    name=f"pv_{h}_{it}")
                    for jc in range(NJ):
                        nc.tensor.matmul(
                            pv,
                            v_sb[:, jc, h, :],
                            pt[:, jc, t, it * T5:(it + 1) * T5],
                            start=(jc == 0), stop=(jc == NJ - 1),
                        )
                    # reciprocal of the denominator row as exp(-ln(d)) on
                    # ScalarE — DVE's exact reciprocal costs 3.3us per row
                    lg = rec_pool.tile([DH + 1, T5], f32, tag="lg",
                                       name=f"lg_{h}_{it}")
                    nc.scalar.activation(
                        out=lg[DH:DH + 1, :], in_=pv[DH:DH + 1, :],
                        func=mybir.ActivationFunctionType.Ln)
                    rec = rec_pool.tile([DH + 1, T5], f32, tag="rec",
                                        name=f"rec_{h}_{it}")
                    nc.scalar.activation(
                        out=rec[DH:DH + 1, :], in_=lg[DH:DH + 1, :],
                        func=Exp, scale=-1.0)
                    # broadcast the reciprocal row across 64 partitions via a
                    # DRAM bounce (step-0 partition APs are DRAM-only)
                    dr = dram_pool.tile([1, T5], f32, tag="dr",
                                        name=f"dr_{h}_{it}")
                    nc.sync.dma_start(out=dr, in_=rec[DH:DH + 1, :])
                    bc = bc_pool.tile([DH, T5], f32, tag="bc",
                                      name=f"bc_{h}_{it}")
                    bsrc = bass.AP(tensor=dr.tensor, offset=dr.offset,
                                   ap=[[0, DH], [1, T5]])
                    nc.sync.dma_start(out=bc, in_=bsrc)
                    if t == 0:
                        nc.vector.tensor_mul(
                            out=outT_sb[0:DH, hp, it * T5:(it + 1) * T5],
                            in0=pv[0:DH, :], in1=bc)
                    else:
                        st = stg_pool.tile([DH, T5], bf16, tag="st",
                                           name=f"st_{h}_{it}")
                        nc.vector.tensor_mul(out=st, in0=pv[0:DH, :], in1=bc)
                        nc.sync.dma_start(
                            out=outT_sb[DH:P, hp, it * T5:(it + 1) * T5],
                            in_=st)
            return emit

        def proj_group(nj):
            def emit():
                with nc.named_scope("proj"):
                    osb = ost_pool.tile([P, C], f32, tag="osb",
                                        name=f"osb_{nj}")
                    for et in range(2):
                        pp = mm_ps.tile([P, T5], f32, tag="mm",
                                        name=f"pp_{nj}_{et}")
                        for kc in range(KC):
                            nc.tensor.matmul(
                                pp[:, :384],
                                outT_sb[:, kc, nj * P:(nj + 1) * P],
                                wp_sb[:, kc, et * 384:(et + 1) * 384],
                                start=(kc == 0), stop=(kc == KC - 1),
                            )
                        nc.vector.tensor_add(
                            out=osb[:, et * 384:(et + 1) * 384],
                            in0=pp[:, :384],
                            in1=bp_sb[:, et * 384:(et + 1) * 384],
                        )
                    nc.sync.dma_start(
                        out=out_d.ap()[nj * P:(nj + 1) * P, :], in_=osb)
            return emit

        # ---- emission: software-pipelined at pair granularity ----
        # Per steady-state pair: scores(hp) -> PV(hp-1) -> qk(hp+1). The PV
        # and qk groups keep the PE busy (and the HAM clock warm) while
        # ScalarE runs exp over pair hp. NOTE: other matmuls must NOT be
        # emitted BETWEEN the row-tiled K=64 score matmuls — a K=64 ldweights
        # hoisted over an in-flight full-K accumulating matmul corrupts its
        # weights. Phase boundaries here are all in the safe direction
        # (full-K ldweights conflict with the foreground tile and stall).

        def scores_phase(hp, pt):
            with nc.named_scope("scores"):
                for jc in range(NJ):
                    e_t = e_pool.tile([P, 2, N], bf16, tag="e",
                                      name=f"e_{hp}_{jc}")
                    nc.sync.dma_start(out=e_t, in_=be_d.ap()[hp, jc])
                    sc = sc_ps.tile([P, 2 * N], f32, tag="sc",
                                    name=f"sc_{hp}_{jc}")
                    for it in range(NT):
                        nc.tensor.matmul(
                            sc[:, 0 * N + it * T5:0 * N + (it + 1) * T5],
                            kT_sb[0:64, hp, jc * P:(jc + 1) * P],
                            qT_sb[0:64, hp, it * T5:(it + 1) * T5],
                            start=True, stop=True,
                        )
                        nc.tensor.matmul(
                            sc[:, 1 * N + it * T5:1 * N + (it + 1) * T5],
                            kT_sb[64:128, hp, jc * P:(jc + 1) * P],
                            qT_sb[64:128, hp, it * T5:(it + 1) * T5],
                            start=True, stop=True,
                        )
                    nc.scalar.activation(
                        out=pt[:, jc],
                        in_=sc[:].rearrange("p (t n) -> p t n", t=2),
                        func=Exp,
                    )
                    nc.vector.tensor_mul(out=pt[:, jc], in0=pt[:, jc], in1=e_t)

        for which in range(2):
            for it in range(NT):
                qk_group(0, which, it)()

        pts = [None] * HP
        for hp in range(HP):
            pts[hp] = pt_pool.tile([P, NJ, 2, N], bf16, tag="pt",
                                   name=f"pt_{hp}")
            scores_phase(hp, pts[hp])
            if hp == 0:
                for nj in range(NJ):
                    for et in range(2):
                        v_group(nj, et)()
            else:
                for t in range(2):
                    for it in range(NT):
                        pv_group(hp - 1, pts[hp - 1], t, it)()
            if hp + 1 < HP:
                for w in range(2):
                    for it in range(NT):
                        qk_group(hp + 1, w, it)()

        for t in range(2):
            for it in range(NT):
                pv_group(HP - 1, pts[HP - 1], t, it)()
        for nj in range(NJ):
            proj_group(nj)()

    nc.finalize()
    return nc


def _get_nc():
    if "nc" not in _BUILT:
        _BUILT["nc"] = _build_nc()
    return _BUILT["nc"]


def _prep_inputs(x, W_qkv, W_proj, b_proj, bias_table, rel_index):
    bf = ml_dtypes.bfloat16
    x = np.asarray(x, dtype=np.float32)
    W_qkv = np.asarray(W_qkv, dtype=np.float32)
    W_proj = np.asarray(W_proj, dtype=np.float32)
    b_proj = np.asarray(b_proj, dtype=np.float32)
    bias_table = np.asarray(bias_table, dtype=np.float32)
    rel_index = np.asarray(rel_index)

    xT = np.ascontiguousarray(x.transpose(0, 2, 1)).astype(bf)       # [B, C, N]
    wq = W_qkv.copy()
    wq[:, :C] *= DH ** -0.5          # fold the attention scale into W_q
    wq = wq.astype(bf)
    wp = W_proj.astype(bf)

    # E[h, j, i] = exp(bias_table[rel_index[i, j], h]); pair-tiled layout
    g = bias_table[rel_index]                      # [i, j, H]
    E = np.exp(g).transpose(2, 1, 0)               # [H, j, i]
    Ep = E.reshape(HP, 2, NJ, P, N).transpose(0, 2, 3, 1, 4)  # [hp, jc, p, t, i]
    Ep = np.ascontiguousarray(Ep).astype(bf)

    shared = {"wqkv": wq, "wproj": wp, "bproj": b_proj, "bexp": Ep}
    in_maps = []
    for b in range(B):
        m = dict(shared)
        m["xT"] = np.ascontiguousarray(xT[b])
        in_maps.append(m)
    return in_maps


def run(x, W_qkv, W_proj, b_proj, bias_table, rel_index, trace=False):
    """Returns (output [B, N, C] f32, exec_time_ns or None)."""
    from concourse.bass_utils import run_bass_kernel_spmd

    nc = _get_nc()
    in_maps = _prep_inputs(x, W_qkv, W_proj, b_proj, bias_table, rel_index)
    res = run_bass_kernel_spmd(nc, in_maps, core_ids=list(range(B)), trace=trace)
    out = np.stack([r["out"] for r in res.results]).astype(np.float32)
    return out, res.exec_time_ns


def kernel(x, W_qkv, W_proj, b_proj, bias_table, rel_index):
    out, _ = run(x, W_qkv, W_proj, b_proj, bias_table, rel_index, trace=False)
    return out


# revision 17
# speedup vs baseline: 1.0938x; 1.0133x over previous
"""Trainium2 Bass kernel for ViT-style multi-head attention with relative
position bias.

Problem (per full input):
  x        [8, 1024, 768] f32
  W_qkv    [768, 2304]    f32
  W_proj   [768, 768]     f32
  b_proj   [768]          f32
  bias_table [2047, 12]   f32
  rel_index  [1024, 1024] int32

Sharding: pure data parallel — one batch element per NeuronCore (B=8 over 8
cores), weights replicated. No collectives.

Per-core kernel (all matmuls bf16, accum f32 in PSUM):
  - host pre-transposes x -> xT [C, N]; qT,kT computed in [d, n] layout,
    v in [n, d] layout — both directly from xT, no on-device transposes.
  - scores computed TRANSPOSED sT[j, i] = kT_chunk^T @ qT so the softmax'd
    matrix pT is already the PV matmul's moving operand. The two heads of a
    pair run concurrently at PE row groups 0/64 (K=64 row tiling).
  - rel-pos bias folded multiplicatively: host precomputes E = exp(biasT);
    kernel does pT = exp(sT) * E (exp on ScalarE over [128, 2048] pair
    tiles, multiply on VectorE).
  - PV: out'T[65, i] = [v|1]^T @ pT — row 64 is the softmax denominator.
  - normalize: reciprocal_approx_fast of the denom row, broadcast across 64
    partitions via a DRAM bounce (step-0 partition APs are DRAM-only),
    multiply during PSUM->SBUF eviction into the outT layout proj needs.
  - proj: lhsT = outT chunks, + broadcast b_proj, DMA out.

Program order interleaves qkv / v / PV matmul groups into the score slots so
the PE never idles while ScalarE runs exp — keeps the HAM clock gate at
2.4 GHz (idle windows drop the PE to 1.2 GHz).
"""

import numpy as np
import ml_dtypes

B = 8
N = 1024
C = 768
H = 12
DH = 64
P = 128
KC = C // P          # 6 contraction chunks of 128 over C
NJ = N // P          # 8 chunks of 128 over the j (key) axis
NT = N // 512        # 2 tiles of 512 over the i (query) axis
HP = H // 2          # 6 head pairs
T5 = 512

_BUILT = {}


def _build_nc():
    from contextlib import ExitStack
    import concourse.bass as bass
    import concourse.mybir as mybir
    import concourse.tile as tile
    from concourse import bacc

    bf16 = mybir.dt.bfloat16
    f32 = mybir.dt.float32
    Exp = mybir.ActivationFunctionType.Exp

    nc = bacc.Bacc("TRN2", target_bir_lowering=False, debug=False)

    xT_d = nc.dram_tensor("xT", [C, N], bf16, kind="ExternalInput")
    w_d = nc.dram_tensor("wqkv", [C, 3 * C], bf16, kind="ExternalInput")
    wp_d = nc.dram_tensor("wproj", [C, C], bf16, kind="ExternalInput")
    bp_d = nc.dram_tensor("bproj", [C], f32, kind="ExternalInput")
    be_d = nc.dram_tensor("bexp", [HP, NJ, P, 2, N], bf16, kind="ExternalInput")
    out_d = nc.dram_tensor("out", [N, C], f32, kind="ExternalOutput")

    with ExitStack() as ctx:
        tc = ctx.enter_context(tile.TileContext(nc))

        singles = ctx.enter_context(tc.tile_pool(name="singles", bufs=1))
        pt_pool = ctx.enter_context(tc.tile_pool(name="pt_pool", bufs=2))
        e_pool = ctx.enter_context(tc.tile_pool(name="e_pool", bufs=3))
        rec_pool = ctx.enter_context(tc.tile_pool(name="rec_pool", bufs=2))
        bc_pool = ctx.enter_context(tc.tile_pool(name="bc_pool", bufs=2))
        stg_pool = ctx.enter_context(tc.tile_pool(name="stg_pool", bufs=2))
        ost_pool = ctx.enter_context(tc.tile_pool(name="ost_pool", bufs=2))
        dram_pool = ctx.enter_context(tc.tile_pool(name="dram_pool", bufs=2, space="DRAM"))
        mm_ps = ctx.enter_context(tc.tile_pool(name="mm_ps", bufs=2, space="PSUM"))
        sc_ps = ctx.enter_context(tc.tile_pool(name="sc_ps", bufs=1, space="PSUM"))
        pv_ps = ctx.enter_context(tc.tile_pool(name="pv_ps", bufs=2, space="PSUM"))

        # ---- resident SBUF tensors (loads chunked so the PE starts early) --
        xT_sb = singles.tile([P, KC, N], bf16)
        xT_r = xT_d.ap().rearrange("(kc p) n -> p kc n", p=P)
        w_sb = singles.tile([P, KC, 3 * C], bf16)
        w_r = w_d.ap().rearrange("(kc p) d -> p kc d", p=P)
        for kc in range(KC):
            nc.sync.dma_start(out=xT_sb[:, kc], in_=xT_r[:, kc])
            nc.sync.dma_start(out=w_sb[:, kc], in_=w_r[:, kc])
        wp_sb = singles.tile([P, KC, C], bf16)
        nc.sync.dma_start(out=wp_sb, in_=wp_d.ap().rearrange("(kc p) d -> p kc d", p=P))
        bp_sb = singles.tile([P, C], f32)
        bp_ap = bp_d.ap()
        bp_bcast = bass.AP(tensor=bp_ap.tensor, offset=bp_ap.offset,
                           ap=[[0, P], *bp_ap.ap])
        nc.gpsimd.dma_start(out=bp_sb, in_=bp_bcast)

        qT_sb = singles.tile([P, KC, N], bf16)   # chunk hp = heads (2hp, 2hp+1)
        kT_sb = singles.tile([P, KC, N], bf16)
        v_sb = singles.tile([P, NJ, H, DH + 1], bf16)  # col DH = ones
        nc.vector.memset(v_sb[:, :, :, DH:DH + 1], 1.0)
        outT_sb = singles.tile([P, KC, N], bf16)

        # ---- matmul group emitters (closures; emitted in interleaved order) --

        def v_group(nj, et):
            e0 = et * 384
            def emit():
                with nc.named_scope("v_mm"):
                    ps = mm_ps.tile([P, T5], f32, tag="mm", name=f"ps_v_{nj}_{et}")
                    for kc in range(KC):
                        nc.tensor.matmul(
                            ps[:, :384],
                            xT_sb[:, kc, nj * P:(nj + 1) * P],
                            w_sb[:, kc, 2 * C + e0:2 * C + e0 + 384],
                            start=(kc == 0), stop=(kc == KC - 1),
                        )
                    h0 = e0 // DH
                    nc.vector.tensor_copy(
                        out=v_sb[:, nj, h0:h0 + 6, 0:DH],
                        in_=ps[:, :384].rearrange("p (h d) -> p h d", h=6),
                    )
            return emit

        def qk_group(hp, which, it):
            col0 = hp * P if which == 0 else C + hp * P
            dst = None
            def emit():
                with nc.named_scope("qk_mm"):
                    d = qT_sb if which == 0 else kT_sb
                    ps = mm_ps.tile([P, T5], f32, tag="mm",
                                    name=f"ps_qk_{hp}_{which}_{it}")
                    for kc in range(KC):
                        nc.tensor.matmul(
                            ps,
                            w_sb[:, kc, col0:col0 + P],
                            xT_sb[:, kc, it * T5:(it + 1) * T5],
                            start=(kc == 0), stop=(kc == KC - 1),
                        )
                    nc.vector.tensor_copy(
                        out=d[:, hp, it * T5:(it + 1) * T5], in_=ps)
            return emit

        def pv_group(hp, pt, t, it):
            def emit():
                with nc.named_scope("pv"):
                    h = 2 * hp + t
                    pv = pv_ps.tile([DH + 1, T5], f32, tag="pv",
                                    name=f"pv_{h}_{it}")
                    for jc in range(NJ):
                        nc.tensor.matmul(
                            pv,
                            v_sb[:, jc, h, :],
                            pt[:, jc, t, it * T5:(it + 1) * T5],
                            start=(jc == 0), stop=(jc == NJ - 1),
                        )
                    # evict the PV result early so the PSUM slot frees fast
                    o_sb = stg_pool.tile([DH, T5], bf16, tag="o",
                                         name=f"o_{h}_{it}")
                    nc.vector.tensor_copy(out=o_sb, in_=pv[0:DH, :])
                    # reciprocal of the denominator row. DVE division costs
                    # ~6.5ns per FREE element (8-slice iterative divide), so
                    # reshape the [1,512] row into [128,4] via a DRAM bounce
                    # first — the reciprocal then costs ~nothing — and
                    # broadcast the result across 64 partitions on the way
                    # back (step-0 partition APs are DRAM-only).
                    drow = rec_pool.tile([DH + 1, T5], f32, tag="den",
                                         name=f"den_{h}_{it}")
                    nc.vector.tensor_copy(
                        out=drow[DH:DH + 1, :], in_=pv[DH:DH + 1, :])
                    dra = dram_pool.tile([1, T5], f32, tag="dra",
                                         name=f"dra_{h}_{it}")
                    nc.sync.dma_start(out=dra, in_=drow[DH:DH + 1, :])
                    dcol = rec_pool.tile([P, 4], f32, tag="dcol",
                                         name=f"dcol_{h}_{it}")
                    nc.sync.dma_start(
                        out=dcol,
                        in_=dra.rearrange("a (p f) -> (a p) f", p=P))
                    rcol = rec_pool.tile([P, 4], f32, tag="rcol",
                                         name=f"rcol_{h}_{it}")
                    nc.vector.reciprocal(out=rcol, in_=dcol)
                    drb = dram_pool.tile([1, T5], f32, tag="drb",
                                         name=f"drb_{h}_{it}")
                    nc.sync.dma_start(
                        out=drb.rearrange("a (p f) -> (a p) f", p=P),
                        in_=rcol)
                    bc = bc_pool.tile([DH, T5], f32, tag="bc",
                                      name=f"bc_{h}_{it}")
                    bsrc = bass.AP(tensor=drb.tensor, offset=drb.offset,
                                   ap=[[0, DH], [1, T5]])
                    nc.sync.dma_start(out=bc, in_=bsrc)
                    if t == 0:
                        nc.vector.tensor_mul(
                            out=outT_sb[0:DH, hp, it * T5:(it + 1) * T5],
                            in0=o_sb, in1=bc)
                    else:
                        st = stg_pool.tile([DH, T5], bf16, tag="st",
                                           name=f"st_{h}_{it}")
                        nc.vector.tensor_mul(out=st, in0=o_sb, in1=bc)
                        nc.sync.dma_start(
                            out=outT_sb[DH:P, hp, it * T5:(it + 1) * T5],
                            in_=st)
            return emit

        def proj_group(nj):
            def emit():
                with nc.named_scope("proj"):
                    osb = ost_pool.tile([P, C], f32, tag="osb",
                                        name=f"osb_{nj}")
                    for et in range(2):
                        pp = mm_ps.tile([P, T5], f32, tag="mm",
                                        name=f"pp_{nj}_{et}")
                        for kc in range(KC):
                            nc.tensor.matmul(
                                pp[:, :384],
                                outT_sb[:, kc, nj * P:(nj + 1) * P],
                                wp_sb[:, kc, et * 384:(et + 1) * 384],
                                start=(kc == 0), stop=(kc == KC - 1),
                            )
                        nc.vector.tensor_add(
                            out=osb[:, et * 384:(et + 1) * 384],
                            in0=pp[:, :384],
                            in1=bp_sb[:, et * 384:(et + 1) * 384],
                        )
                    nc.sync.dma_start(
                        out=out_d.ap()[nj * P:(nj + 1) * P, :], in_=osb)
            return emit

        # ---- emission: software-pipelined at pair granularity ----
        # Per steady-state pair: scores(hp) -> PV(hp-1) -> qk(hp+1). The PV
        # and qk groups keep the PE busy (and the HAM clock warm) while
        # ScalarE runs exp over pair hp. NOTE: other matmuls must NOT be
        # emitted BETWEEN the row-tiled K=64 score matmuls — a K=64 ldweights
        # hoisted over an in-flight full-K accumulating matmul corrupts its
        # weights. Phase boundaries here are all in the safe direction
        # (full-K ldweights conflict with the foreground tile and stall).

        def scores_phase(hp, pt):
            with nc.named_scope("scores"):
                for jc in range(NJ):
                    e_t = e_pool.tile([P, 2, N], bf16, tag="e",
                                      name=f"e_{hp}_{jc}")
                    nc.sync.dma_start(out=e_t, in_=be_d.ap()[hp, jc])
                    sc = sc_ps.tile([P, 2 * N], f32, tag="sc",
                                    name=f"sc_{hp}_{jc}")
                    for it in range(NT):
                        nc.tensor.matmul(
                            sc[:, 0 * N + it * T5:0 * N + (it + 1) * T5],
                            kT_sb[0:64, hp, jc * P:(jc + 1) * P],
                            qT_sb[0:64, hp, it * T5:(it + 1) * T5],
                            start=True, stop=True,
                        )
                        nc.tensor.matmul(
                            sc[:, 1 * N + it * T5:1 * N + (it + 1) * T5],
                            kT_sb[64:128, hp, jc * P:(jc + 1) * P],
                            qT_sb[64:128, hp, it * T5:(it + 1) * T5],
                            start=True, stop=True,
                        )
                    # exp into a scratch tile; the E-multiply then has three
                    # distinct all-SBUF bf16 operands (DVE fast-mode eligible)
                    es = e_pool.tile([P, 2, N], bf16, tag="es",
                                     name=f"es_{hp}_{jc}")
                    nc.scalar.activation(
                        out=es,
                        in_=sc[:].rearrange("p (t n) -> p t n", t=2),
                        func=Exp,
                    )
                    nc.vector.tensor_mul(out=pt[:, jc], in0=es, in1=e_t)

        for which in range(2):
            for it in range(NT):
                qk_group(0, which, it)()

        pts = [None] * HP
        for hp in range(HP):
            pts[hp] = pt_pool.tile([P, NJ, 2, N], bf16, tag="pt",
                                   name=f"pt_{hp}")
            scores_phase(hp, pts[hp])
            if hp == 0:
                for nj in range(NJ):
                    for et in range(2):
                        v_group(nj, et)()
            else:
                for t in range(2):
                    for it in range(NT):
                        pv_group(hp - 1, pts[hp - 1], t, it)()
            if hp + 1 < HP:
                for w in range(2):
                    for it in range(NT):
                        qk_group(hp + 1, w, it)()

        for t in range(2):
            for it in range(NT):
                pv_group(HP - 1, pts[HP - 1], t, it)()
        for nj in range(NJ):
            proj_group(nj)()

    nc.finalize()
    return nc


def _get_nc():
    if "nc" not in _BUILT:
        _BUILT["nc"] = _build_nc()
    return _BUILT["nc"]


def _prep_inputs(x, W_qkv, W_proj, b_proj, bias_table, rel_index):
    bf = ml_dtypes.bfloat16
    x = np.asarray(x, dtype=np.float32)
    W_qkv = np.asarray(W_qkv, dtype=np.float32)
    W_proj = np.asarray(W_proj, dtype=np.float32)
    b_proj = np.asarray(b_proj, dtype=np.float32)
    bias_table = np.asarray(bias_table, dtype=np.float32)
    rel_index = np.asarray(rel_index)

    xT = np.ascontiguousarray(x.transpose(0, 2, 1)).astype(bf)       # [B, C, N]
    wq = W_qkv.copy()
    wq[:, :C] *= DH ** -0.5          # fold the attention scale into W_q
    wq = wq.astype(bf)
    wp = W_proj.astype(bf)

    # E[h, j, i] = exp(bias_table[rel_index[i, j], h]); pair-tiled layout
    g = bias_table[rel_index]                      # [i, j, H]
    E = np.exp(g).transpose(2, 1, 0)               # [H, j, i]
    Ep = E.reshape(HP, 2, NJ, P, N).transpose(0, 2, 3, 1, 4)  # [hp, jc, p, t, i]
    Ep = np.ascontiguousarray(Ep).astype(bf)

    shared = {"wqkv": wq, "wproj": wp, "bproj": b_proj, "bexp": Ep}
    in_maps = []
    for b in range(B):
        m = dict(shared)
        m["xT"] = np.ascontiguousarray(xT[b])
        in_maps.append(m)
    return in_maps


def run(x, W_qkv, W_proj, b_proj, bias_table, rel_index, trace=False):
    """Returns (output [B, N, C] f32, exec_time_ns or None)."""
    from concourse.bass_utils import run_bass_kernel_spmd

    nc = _get_nc()
    in_maps = _prep_inputs(x, W_qkv, W_proj, b_proj, bias_table, rel_index)
    res = run_bass_kernel_spmd(nc, in_maps, core_ids=list(range(B)), trace=trace)
    out = np.stack([r["out"] for r in res.results]).astype(np.float32)
    return out, res.exec_time_ns


def kernel(x, W_qkv, W_proj, b_proj, bias_table, rel_index):
    out, _ = run(x, W_qkv, W_proj, b_proj, bias_table, rel_index, trace=False)
    return out
